# revision 14
# baseline (speedup 1.0000x reference)
"""DeepFM forward on 8 Trainium2 NeuronCores (Bass/Tile, SPMD).

Strategy: data-parallel over the batch (2048 rows/core), embedding tables
replicated. The first-order table, second-order tables, and a precomputed
per-row sum-of-squares column are fused host-side into one [F_CAT*V, 66]
fp16 table; per (batch-tile, feature) indirect DMAs gather 66-wide rows.
Gathered rows (+ cont features and their squares) are DMA-transposed into
X.T; the MLP weight matrix is row-permuted to match, with cont rows folded
through cont_t2.

All FM reductions run in column space on the tensor engine via a selection
matmul stack (s vector, first-order total, sum-of-squares total come out as
a [66, N] PSUM block per batch n-tile), so the gather chain on GpSimd is the
only serial bottleneck; transposes, FM, and layer-1 matmuls overlap it.

MLP runs in fp16 (fp32 accumulation in PSUM); batchnorm statistics are
exchanged with two tiny AllReduces. Output is assembled in column space
([2, Bc] probabilities) and unsharded host-side.
"""

import numpy as np

# ---- problem constants (hardcoded per harness contract) ----
B, F_CAT, F_CONT, V, D = 16384, 26, 13, 100000, 64
H1, H2 = 1024, 512
N_CORES = 8
BN_EPS = 1e-5

CFG_FULL = dict(B=B, V=V, n_cores=N_CORES)

_P = 128
_EW = D + 2            # 66: 64 emb cols + first-order col + row-sumsq col
_RWG = F_CAT * _EW     # 1716 gathered cols per batch row
_RWF = 1792            # padded row width = 14 * 128
_CFO = _RWG            # cont cols at 1716..1728
_CFE = _CFO + F_CONT   # 1729
_CQO = _CFE            # cont-squared cols at 1729..1741
_CQE = _CQO + F_CONT   # 1742


def _build_program(cfg):
    """Build the per-core SPMD Bass program. Returns nc."""
    import concourse.bacc as bacc
    import concourse.bass as bass
    import concourse.mybir as mybir
    import concourse.tile as tile

    F32, FP16, I32 = mybir.dt.float32, mybir.dt.float16, mybir.dt.int32
    AF = mybir.ActivationFunctionType
    OP = mybir.AluOpType
    AX = mybir.AxisListType
    P = _P

    ncore = cfg["n_cores"]
    Bfull = cfg["B"]
    Vv = cfg["V"]
    Bc = Bfull // ncore          # batch rows per core
    TB = Bc // P                 # batch tiles per core
    NB = min(256, Bc)            # matmul moving free dim
    NN = Bc // NB                # batch n-tiles
    TPN = NB // P                # 128-tiles per n-tile
    NKC = _RWF // P              # K chunks (14)
    NM1 = H1 // P                # 8
    NM2 = H2 // P                # 4
    rg = [list(range(ncore))]

    NQ = cfg.get("swdge_queues", 4)
    nc = bacc.Bacc(num_devices=ncore, num_swdge_queues=NQ)

    idxT = nc.dram_tensor("idxT", [P, TB * F_CAT], I32, kind="ExternalInput")
    cfT = nc.dram_tensor("cfT", [P, TB * F_CONT], FP16, kind="ExternalInput")
    bigt = nc.dram_tensor("bigt", [F_CAT * Vv, _EW], FP16, kind="ExternalInput")
    w1 = nc.dram_tensor("w1", [_RWF, H1], FP16, kind="ExternalInput")
    wsel = nc.dram_tensor("wsel", [_RWF, _EW], FP16, kind="ExternalInput")
    w2 = nc.dram_tensor("w2", [H1, H2], FP16, kind="ExternalInput")
    w3 = nc.dram_tensor("w3", [P, NM2], FP16, kind="ExternalInput")
    bnp = nc.dram_tensor("bnp", [P, 3 * NM1 + 3 * NM2 + 1], F32, kind="ExternalInput")
    ident = nc.dram_tensor("ident", [P, P], FP16, kind="ExternalInput")
    out = nc.dram_tensor("out", [2, Bc], F32, kind="ExternalOutput")

    with tile.TileContext(nc) as tc:
        with (
            tc.tile_pool(name="const", bufs=1) as cpool,
            tc.tile_pool(name="big", bufs=1) as bpool,
            tc.tile_pool(name="s2p", bufs=2) as s2pool,
            tc.tile_pool(name="psmm", bufs=3, space="PSUM") as psmm,
            tc.tile_pool(name="psel", bufs=1, space="PSUM") as psel,
            tc.tile_pool(name="pz", bufs=1, space="PSUM") as pz,
            tc.tile_pool(name="ptt", bufs=2, space="PSUM") as ptt,
            tc.tile_pool(name="dram", bufs=1, space="DRAM") as dpool,
        ):
            # ---- constants (batch-dependent inputs first on the sync queue
            # so gathers can start immediately; weights trickle in on the
            # scalar hwdge queue) ----
            idx_sb = cpool.tile([P, TB * F_CAT], I32, tag="idxT")
            nc.sync.dma_start(out=idx_sb[:, 0 : 2 * F_CAT], in_=idxT[:, 0 : 2 * F_CAT])
            nc.sync.dma_start(
                out=idx_sb[:, 2 * F_CAT :], in_=idxT[:, 2 * F_CAT :]
            )
            cf_sb = cpool.tile([P, TB * F_CONT], FP16, tag="cfT")
            nc.sync.dma_start(out=cf_sb[:], in_=cfT[:])
            bnsb = cpool.tile([P, 3 * NM1 + 3 * NM2 + 1], F32, tag="bnp")
            nc.sync.dma_start(out=bnsb[:], in_=bnp[:])
            w1sb = []
            for k in range(NKC):
                t = cpool.tile([P, H1], FP16, tag=f"w1_{k}")
                nc.scalar.dma_start(out=t[:], in_=w1[k * P : (k + 1) * P, :])
                w1sb.append(t)
            wselsb = []
            for k in range(NKC):
                t = cpool.tile([P, _EW], FP16, tag=f"wsel_{k}")
                nc.scalar.dma_start(out=t[:], in_=wsel[k * P : (k + 1) * P, :])
                wselsb.append(t)
            w2sb = []
            for k in range(NM1):
                t = cpool.tile([P, H2], FP16, tag=f"w2_{k}")
                nc.scalar.dma_start(out=t[:], in_=w2[k * P : (k + 1) * P, :])
                w2sb.append(t)
            w3sb = cpool.tile([P, NM2], FP16, tag="w3")
            nc.scalar.dma_start(out=w3sb[:], in_=w3[:])
            eps_t = cpool.tile([P, 1], F32, tag="eps")
            nc.vector.memset(eps_t[:], BN_EPS)
            halves = cpool.tile([D, 1], FP16, tag="halves")
            nc.vector.memset(halves[:], 0.5)
            identsb = cpool.tile([P, P], FP16, tag="ident")
            nc.sync.dma_start(out=identsb[:], in_=ident[:])

            b1c = bnsb[:, 0:NM1]
            g1c = bnsb[:, NM1 : 2 * NM1]
            be1c = bnsb[:, 2 * NM1 : 3 * NM1]
            o2 = 3 * NM1
            b2c = bnsb[:, o2 : o2 + NM2]
            g2c = bnsb[:, o2 + NM2 : o2 + 2 * NM2]
            be2c = bnsb[:, o2 + 2 * NM2 : o2 + 3 * NM2]
            bias_col = bnsb[:, o2 + 3 * NM2 : o2 + 3 * NM2 + 1]

            # cont squared features, all tiles at once
            cfsq = cpool.tile([P, TB * F_CONT], FP16, tag="cfsq")
            nc.vector.tensor_tensor(out=cfsq[:], in0=cf_sb[:], in1=cf_sb[:], op=OP.mult)

            # ---- persistent activations ----
            xtn = [
                bpool.tile([P, NKC, NB], FP16, tag=f"xtn_{n}", name=f"xtn_{n}")
                for n in range(NN)
            ]
            h1t = [bpool.tile([P, Bc], FP16, tag=f"h1_{m}", name=f"h1_{m}") for m in range(NM1)]
            h2t = [bpool.tile([P, Bc], FP16, tag=f"h2_{m}", name=f"h2_{m}") for m in range(NM2)]

            acc1 = bpool.tile([P, NM1 * NN], F32, tag="acc1")
            acc1s = bpool.tile([P, NM1 * NN], F32, tag="acc1s")
            acc2 = bpool.tile([P, NM2 * NN], F32, tag="acc2")
            acc2s = bpool.tile([P, NM2 * NN], F32, tag="acc2s")
            scrh = bpool.tile([P, NB], FP16, tag="scrh")
            fmsb = bpool.tile([1, Bc], F32, tag="fmsb")
            zrow = bpool.tile([1, Bc], F32, tag="zrow")
            outp = bpool.tile([1, Bc], F32, tag="outp")
            outn = bpool.tile([1, Bc], F32, tag="outn")

            # gather row buffers; pad+cont regions zeroed once (pad columns
            # hit zero weight rows, but must stay finite for fp16 matmul)
            NRB = 6
            rows_bufs = [
                bpool.tile([P, _RWF], FP16, tag=f"rows{j}", name=f"rows{j}")
                for j in range(NRB)
            ]
            for j in range(NRB):
                nc.vector.memset(rows_bufs[j][:, _RWG:_RWF], 0.0)

            # ---- gather + transpose + per-n-tile compute, interleaved ----
            gq = [0]

            def emit_tile(t):
                rows = rows_bufs[t % NRB]
                for f in range(F_CAT):
                    inst = nc.gpsimd.indirect_dma_start(
                        out=rows[:, f * _EW : (f + 1) * _EW],
                        out_offset=None,
                        in_=bigt[:],
                        in_offset=bass.IndirectOffsetOnAxis(
                            ap=idx_sb[:, t * F_CAT + f : t * F_CAT + f + 1], axis=0
                        ),
                    )
                    if NQ > 1:
                        inst.ins.queue = f"qPoolDynamic{(gq[0] % NQ) or ''}"
                        gq[0] += 1
                nc.vector.tensor_copy(
                    out=rows[:, _CFO:_CFE],
                    in_=cf_sb[:, t * F_CONT : (t + 1) * F_CONT],
                )
                nc.vector.tensor_copy(
                    out=rows[:, _CQO:_CQE],
                    in_=cfsq[:, t * F_CONT : (t + 1) * F_CONT],
                )
                n, tp = t // TPN, t % TPN
                for k in range(NKC):
                    pst = ptt.tile([P, P], FP16, tag="tt")
                    nc.tensor.transpose(
                        out=pst[:], in_=rows[:, k * P : (k + 1) * P], identity=identsb[:]
                    )
                    dst = xtn[n][:, k, tp * P : (tp + 1) * P]
                    if k % 2 == 0:
                        nc.scalar.activation(out=dst, in_=pst[:], func=AF.Copy)
                    else:
                        nc.vector.tensor_copy(out=dst, in_=pst[:])

            def emit_l1(n):
                # layer-1 matmuls for this n-tile
                for m in range(NM1):
                    ps = psmm.tile([P, NB], F32, tag="mm")
                    for k in range(NKC):
                        nc.tensor.matmul(
                            out=ps[:],
                            lhsT=w1sb[k][:, m * P : (m + 1) * P],
                            rhs=xtn[n][:, k, :],
                            start=(k == 0),
                            stop=(k == NKC - 1),
                        )
                    j = m * NN + n
                    nc.scalar.activation(
                        out=h1t[m][:, n * NB : (n + 1) * NB], in_=ps[:],
                        func=AF.Identity, bias=b1c[:, m : m + 1],
                    )
                    nc.vector.tensor_reduce(
                        out=acc1[:, j : j + 1], in_=ps[:], axis=AX.X, op=OP.add
                    )
                    nc.vector.tensor_tensor(
                        out=scrh[:],
                        in0=h1t[m][:, n * NB : (n + 1) * NB],
                        in1=h1t[m][:, n * NB : (n + 1) * NB],
                        op=OP.mult,
                    )
                    nc.vector.tensor_reduce(
                        out=acc1s[:, j : j + 1], in_=scrh[:], axis=AX.X, op=OP.add
                    )

            def emit_fm(n):
                # FM selection stack: SEL = s(64) x NB (cont folded via wsel)
                sel = psel.tile([D, NB], F32, tag="sel")
                for k in range(NKC):
                    nc.tensor.matmul(
                        out=sel[:],
                        lhsT=wselsb[k][:, 0:D],
                        rhs=xtn[n][:, k, :],
                        start=(k == 0),
                        stop=(k == NKC - 1),
                    )
                s2t = s2pool.tile([D, NB], FP16, tag="s2")
                nc.scalar.activation(out=s2t[:], in_=sel[:], func=AF.Square)
                # zfm = (first_total - 0.5*qsum) + 0.5*sum_d s^2, all as M=1
                # matmuls accumulating at PSUM partition 0
                zfm = pz.tile([1, NB], F32, tag="zfm")
                for k in range(NKC):
                    nc.tensor.matmul(
                        out=zfm[:],
                        lhsT=wselsb[k][:, D : D + 1],
                        rhs=xtn[n][:, k, :],
                        start=(k == 0),
                        stop=False,
                    )
                nc.tensor.matmul(
                    out=zfm[:], lhsT=halves[:], rhs=s2t[:], start=False, stop=True
                )
                nsl = slice(n * NB, (n + 1) * NB)
                nc.vector.tensor_copy(out=fmsb[0:1, nsl], in_=zfm[:])

            for t in range(TB):
                emit_tile(t)
                if t % TPN == TPN - 1:
                    n = t // TPN
                    emit_l1(n)
                    if n < NN - 3:
                        emit_fm(n)

            # ---- BN1 stats ----
            st1 = bpool.tile([P, 2 * NM1], F32, tag="st1")
            nc.vector.tensor_reduce(
                out=st1[:, :NM1],
                in_=acc1[:].rearrange("p (m n) -> p m n", n=NN),
                axis=AX.X, op=OP.add,
            )
            nc.vector.tensor_reduce(
                out=st1[:, NM1:],
                in_=acc1s[:].rearrange("p (m n) -> p m n", n=NN),
                axis=AX.X, op=OP.add,
            )
            st1i = dpool.tile([P, 2 * NM1], F32, tag="st1i")
            st1o = dpool.tile([P, 2 * NM1], F32, tag="st1o")
            nc.gpsimd.dma_start(out=st1i[:], in_=st1[:])
            nc.gpsimd.collective_compute(
                "AllReduce", OP.add, replica_groups=rg,
                ins=[st1i[:].opt()], outs=[st1o[:].opt()],
            )
            emit_fm(NN - 3)
            emit_fm(NN - 2)
            gst1 = bpool.tile([P, 2 * NM1], F32, tag="gst1")
            nc.gpsimd.dma_start(out=gst1[:], in_=st1o[:])

            mu1 = bpool.tile([P, NM1], F32, tag="mu1")
            var1 = bpool.tile([P, NM1], F32, tag="var1")
            a1 = bpool.tile([P, NM1], F32, tag="a1")
            bp1 = bpool.tile([P, NM1], F32, tag="bp1")
            inv_b = 1.0 / Bfull
            nc.vector.tensor_scalar(
                out=mu1[:], in0=gst1[:, :NM1], scalar1=inv_b, scalar2=None, op0=OP.mult
            )
            nc.vector.tensor_tensor(out=var1[:], in0=mu1[:], in1=mu1[:], op=OP.mult)
            nc.vector.tensor_scalar(
                out=a1[:], in0=gst1[:, NM1:], scalar1=inv_b, scalar2=None, op0=OP.mult
            )
            nc.vector.tensor_tensor(out=var1[:], in0=a1[:], in1=var1[:], op=OP.subtract)
            nc.scalar.activation(
                out=var1[:], in_=var1[:], func=AF.Sqrt, bias=eps_t[:, 0:1]
            )
            nc.vector.reciprocal(out=var1[:], in_=var1[:])
            nc.vector.tensor_tensor(out=a1[:], in0=g1c, in1=var1[:], op=OP.mult)
            nc.vector.tensor_tensor(out=bp1[:], in0=mu1[:], in1=a1[:], op=OP.mult)
            nc.vector.tensor_tensor(out=bp1[:], in0=be1c, in1=bp1[:], op=OP.subtract)

            # ---- relu1 + layer 2, pipelined per n ----
            for n in range(NN):
                for m in range(NM1):
                    nc.scalar.activation(
                        out=h1t[m][:, n * NB : (n + 1) * NB],
                        in_=h1t[m][:, n * NB : (n + 1) * NB],
                        func=AF.Relu,
                        scale=a1[:, m : m + 1], bias=bp1[:, m : m + 1],
                    )
                for m in range(NM2):
                    ps = psmm.tile([P, NB], F32, tag="mm")
                    for k in range(NM1):
                        nc.tensor.matmul(
                            out=ps[:],
                            lhsT=w2sb[k][:, m * P : (m + 1) * P],
                            rhs=h1t[k][:, n * NB : (n + 1) * NB],
                            start=(k == 0),
                            stop=(k == NM1 - 1),
                        )
                    j = m * NN + n
                    nc.scalar.activation(
                        out=h2t[m][:, n * NB : (n + 1) * NB], in_=ps[:],
                        func=AF.Identity, bias=b2c[:, m : m + 1],
                    )
                    nc.vector.tensor_reduce(
                        out=acc2[:, j : j + 1], in_=ps[:], axis=AX.X, op=OP.add
                    )
                    nc.vector.tensor_tensor(
                        out=scrh[:],
                        in0=h2t[m][:, n * NB : (n + 1) * NB],
                        in1=h2t[m][:, n * NB : (n + 1) * NB],
                        op=OP.mult,
                    )
                    nc.vector.tensor_reduce(
                        out=acc2s[:, j : j + 1], in_=scrh[:], axis=AX.X, op=OP.add
                    )

            # ---- BN2 ----
            st2 = bpool.tile([P, 2 * NM2], F32, tag="st2")
            nc.vector.tensor_reduce(
                out=st2[:, :NM2],
                in_=acc2[:].rearrange("p (m n) -> p m n", n=NN),
                axis=AX.X, op=OP.add,
            )
            nc.vector.tensor_reduce(
                out=st2[:, NM2:],
                in_=acc2s[:].rearrange("p (m n) -> p m n", n=NN),
                axis=AX.X, op=OP.add,
            )
            st2i = dpool.tile([P, 2 * NM2], F32, tag="st2i")
            st2o = dpool.tile([P, 2 * NM2], F32, tag="st2o")
            nc.gpsimd.dma_start(out=st2i[:], in_=st2[:])
            nc.gpsimd.collective_compute(
                "AllReduce", OP.add, replica_groups=rg,
                ins=[st2i[:].opt()], outs=[st2o[:].opt()],
            )
            emit_fm(NN - 1)
            gst2 = bpool.tile([P, 2 * NM2], F32, tag="gst2")
            nc.gpsimd.dma_start(out=gst2[:], in_=st2o[:])

            mu2 = bpool.tile([P, NM2], F32, tag="mu2")
            var2 = bpool.tile([P, NM2], F32, tag="var2")
            a2 = bpool.tile([P, NM2], F32, tag="a2")
            bp2 = bpool.tile([P, NM2], F32, tag="bp2")
            nc.vector.tensor_scalar(
                out=mu2[:], in0=gst2[:, :NM2], scalar1=inv_b, scalar2=None, op0=OP.mult
            )
            nc.vector.tensor_tensor(out=var2[:], in0=mu2[:], in1=mu2[:], op=OP.mult)
            nc.vector.tensor_scalar(
                out=a2[:], in0=gst2[:, NM2:], scalar1=inv_b, scalar2=None, op0=OP.mult
            )
            nc.vector.tensor_tensor(out=var2[:], in0=a2[:], in1=var2[:], op=OP.subtract)
            nc.scalar.activation(
                out=var2[:], in_=var2[:], func=AF.Sqrt, bias=eps_t[:, 0:1]
            )
            nc.vector.reciprocal(out=var2[:], in_=var2[:])
            nc.vector.tensor_tensor(out=a2[:], in0=g2c, in1=var2[:], op=OP.mult)
            nc.vector.tensor_tensor(out=bp2[:], in0=mu2[:], in1=a2[:], op=OP.mult)
            nc.vector.tensor_tensor(out=bp2[:], in0=be2c, in1=bp2[:], op=OP.subtract)

            # ---- relu2 + layer 3 + sigmoid + output, per n ----
            for n in range(NN):
                for m in range(NM2):
                    nc.scalar.activation(
                        out=h2t[m][:, n * NB : (n + 1) * NB],
                        in_=h2t[m][:, n * NB : (n + 1) * NB],
                        func=AF.Relu,
                        scale=a2[:, m : m + 1], bias=bp2[:, m : m + 1],
                    )
                zz = pz.tile([1, NB], F32, tag="zz")
                for c in range(NM2):
                    nc.tensor.matmul(
                        out=zz[:],
                        lhsT=w3sb[:, c : c + 1],
                        rhs=h2t[c][:, n * NB : (n + 1) * NB],
                        start=(c == 0),
                        stop=(c == NM2 - 1),
                    )
                nsl = slice(n * NB, (n + 1) * NB)
                nc.vector.tensor_tensor(
                    out=zrow[0:1, nsl], in0=zz[:], in1=fmsb[0:1, nsl], op=OP.add
                )
            nc.scalar.activation(
                out=outp[:], in_=zrow[:],
                func=AF.Sigmoid, bias=bias_col[0:1, :],
            )
            nc.vector.tensor_scalar(
                out=outn[:], in0=outp[:], scalar1=-1.0, scalar2=1.0,
                op0=OP.mult, op1=OP.add,
            )
            nc.sync.dma_start(out=out[1:2, :], in_=outp[:])
            nc.sync.dma_start(out=out[0:1, :], in_=outn[:])

    return nc


def _prep_shared(inputs, cfg):
    """Host-side parameter prep (batch-independent). Returns dict of arrays
    shared by all cores."""
    Vv = cfg["V"]
    f32 = np.float32
    f16 = np.float16
    cat_t1 = np.asarray(inputs["cat_t1"], f32)          # [26, V]
    cat_t2 = np.asarray(inputs["cat_t2"], f32)          # [26, V, 64]
    cont_t1 = np.asarray(inputs["cont_t1"], f32)        # [13]
    cont_t2 = np.asarray(inputs["cont_t2"], f32)        # [13, 64]
    W1 = np.asarray(inputs["W1"], f32)                  # [2496, 1024]
    W2 = np.asarray(inputs["W2"], f32)
    W3 = np.asarray(inputs["W3"], f32)                  # [512, 1]
    b1 = np.asarray(inputs["b1"], f32)
    g1 = np.asarray(inputs["g1"], f32)
    be1 = np.asarray(inputs["be1"], f32)
    b2 = np.asarray(inputs["b2"], f32)
    g2 = np.asarray(inputs["g2"], f32)
    be2 = np.asarray(inputs["be2"], f32)
    b3 = np.asarray(inputs["b3"], f32)
    bias = np.asarray(inputs["bias"], f32)

    t2f = cat_t2.reshape(F_CAT * Vv, D).astype(f16)
    bigt = np.empty((F_CAT * Vv, _EW), f16)
    bigt[:, :D] = t2f
    bigt[:, D] = cat_t1.reshape(F_CAT * Vv)
    # row sum-of-squares of the fp16 embeddings (matches device arithmetic)
    bigt[:, D + 1] = (t2f.astype(f32) ** 2).sum(axis=1)

    ncat = F_CAT * D  # 1664
    W1eff = np.einsum("fd,fdh->fh", cont_t2, W1[ncat:].reshape(F_CONT, D, H1))
    # permute W1 rows to the gathered-row layout k' = f*66 + e; t1/sumsq and
    # cont-squared rows are zero, cont rows folded through cont_t2
    w1p = np.zeros((_RWF, H1), f32)
    w1p[:_RWG].reshape(F_CAT, _EW, H1)[:, :D, :] = W1[:ncat].reshape(F_CAT, D, H1)
    w1p[_CFO:_CFE] = W1eff

    # FM selection matrix: cols 0..63 give s = sum_f E (cont folded via
    # cont_t2); col 64 gives the linear fm part first_total - 0.5*qsum
    wselp = np.zeros((_RWF, _EW), f32)
    wv = wselp[:_RWG].reshape(F_CAT, _EW, _EW)
    for e in range(D):
        wv[:, e, e] = 1.0
    wv[:, D, D] = 1.0           # first-order totals
    wv[:, D + 1, D] = -0.5      # -0.5 * sum-of-squares totals
    wselp[_CFO:_CFE, :D] = cont_t2          # s_cont = cf @ cont_t2
    wselp[_CFO:_CFE, D] = cont_t1           # first-order cont
    wselp[_CQO:_CQE, D] = -0.5 * (cont_t2**2).sum(axis=1)  # -0.5 * qct

    NM1n, NM2n = H1 // _P, H2 // _P
    bnpa = np.zeros((_P, 3 * NM1n + 3 * NM2n + 1), f32)
    bnpa[:, 0:NM1n] = b1.reshape(NM1n, _P).T
    bnpa[:, NM1n : 2 * NM1n] = g1.reshape(NM1n, _P).T
    bnpa[:, 2 * NM1n : 3 * NM1n] = be1.reshape(NM1n, _P).T
    o2 = 3 * NM1n
    bnpa[:, o2 : o2 + NM2n] = b2.reshape(NM2n, _P).T
    bnpa[:, o2 + NM2n : o2 + 2 * NM2n] = g2.reshape(NM2n, _P).T
    bnpa[:, o2 + 2 * NM2n : o2 + 3 * NM2n] = be2.reshape(NM2n, _P).T
    bnpa[:, o2 + 3 * NM2n] = float(bias[0]) + float(b3[0])

    return {
        "ident": np.eye(_P, dtype=f16),
        "bigt": bigt,
        "w1": w1p.astype(f16),
        "wsel": wselp.astype(f16),
        "w2": W2.astype(f16),
        "w3": W3[:, 0].reshape(NM2n, _P).T.astype(f16).copy(),
        "bnp": bnpa,
    }


def _prep_in_maps(inputs, cfg):
    """Build the per-core input maps (shard batch, replicate params)."""
    ncore = cfg["n_cores"]
    Vv = cfg["V"]
    Bc = cfg["B"] // ncore
    TB = Bc // _P
    shared = _prep_shared(inputs, cfg)
    cat = np.asarray(inputs["cat_feats"]).astype(np.int32)
    cont = np.asarray(inputs["cont_feats"], np.float32).astype(np.float16)
    idxg = cat + (np.arange(F_CAT, dtype=np.int32) * Vv)[None, :]
    in_maps = []
    for c in range(ncore):
        m = dict(shared)
        # transpose batch-sharded inputs to [128, TB*F] (partition-contiguous)
        ic = idxg[c * Bc : (c + 1) * Bc].reshape(TB, _P, F_CAT)
        m["idxT"] = np.ascontiguousarray(ic.transpose(1, 0, 2)).reshape(_P, TB * F_CAT)
        cc = cont[c * Bc : (c + 1) * Bc].reshape(TB, _P, F_CONT)
        m["cfT"] = np.ascontiguousarray(cc.transpose(1, 0, 2)).reshape(_P, TB * F_CONT)
        in_maps.append(m)
    return in_maps


def _unshard(results, cfg):
    ncore = cfg["n_cores"]
    outs = []
    for c in range(ncore):
        a = results[c]["out"]  # [2, Bc]; column b = batch row b of the shard
        outs.append(np.stack([a[0], a[1]], axis=1))
    return np.concatenate(outs, axis=0)


_CACHE = {}


def _get_program(cfg_key):
    if cfg_key not in _CACHE:
        cfg = dict(B=cfg_key[0], V=cfg_key[1], n_cores=cfg_key[2])
        nc = _build_program(cfg)
        nc.finalize()
        _CACHE[cfg_key] = nc
    return _CACHE[cfg_key]


def run(inputs, trace=False, cfg=None):
    from concourse import bass_utils

    cfg = cfg or CFG_FULL
    nc = _get_program((cfg["B"], cfg["V"], cfg["n_cores"]))
    in_maps = _prep_in_maps(inputs, cfg)
    res = bass_utils.run_bass_kernel_spmd(
        nc, in_maps, core_ids=list(range(cfg["n_cores"])), trace=trace
    )
    return _unshard(res.results, cfg), res


def kernel(**inputs) -> np.ndarray:
    out, _ = run(inputs, trace=False)
    return out


# revision 15
# speedup vs baseline: 1.3862x; 1.3862x over previous
"""DeepFM forward on 8 Trainium2 NeuronCores (Bass/Tile, SPMD).

Strategy: data-parallel over the batch (2048 rows/core), embedding tables
replicated. The first-order table, second-order tables, and a precomputed
per-row sum-of-squares column are fused host-side into one [F_CAT*V, 66]
fp16 table; per (batch-tile, feature) indirect DMAs gather 66-wide rows.
Gathered rows (+ cont features and their squares) are DMA-transposed into
X.T; the MLP weight matrix is row-permuted to match, with cont rows folded
through cont_t2.

All FM reductions run in column space on the tensor engine via a selection
matmul stack (s vector, first-order total, sum-of-squares total come out as
a [66, N] PSUM block per batch n-tile), so the gather chain on GpSimd is the
only serial bottleneck; transposes, FM, and layer-1 matmuls overlap it.

MLP runs in fp16 (fp32 accumulation in PSUM); batchnorm statistics are
exchanged with two tiny AllReduces. Output is assembled in column space
([2, Bc] probabilities) and unsharded host-side.
"""

import numpy as np

# ---- problem constants (hardcoded per harness contract) ----
B, F_CAT, F_CONT, V, D = 16384, 26, 13, 100000, 64
H1, H2 = 1024, 512
N_CORES = 8
BN_EPS = 1e-5

CFG_FULL = dict(B=B, V=V, n_cores=N_CORES)

_P = 128
_EW = D + 2            # 66: 64 emb cols + first-order col + row-sumsq col
_RWG = F_CAT * _EW     # 1716 gathered cols per batch row
_RWF = 1792            # padded row width = 14 * 128
_CFO = _RWG            # cont cols at 1716..1728
_CFE = _CFO + F_CONT   # 1729
_CQO = _CFE            # cont-squared cols at 1729..1741
_CQE = _CQO + F_CONT   # 1742


def _build_program(cfg):
    """Build the per-core SPMD Bass program. Returns nc."""
    import concourse.bacc as bacc
    import concourse.bass as bass
    import concourse.mybir as mybir
    import concourse.tile as tile

    F32, FP16, I32 = mybir.dt.float32, mybir.dt.float16, mybir.dt.int32
    AF = mybir.ActivationFunctionType
    OP = mybir.AluOpType
    AX = mybir.AxisListType
    P = _P

    ncore = cfg["n_cores"]
    Bfull = cfg["B"]
    Vv = cfg["V"]
    Bc = Bfull // ncore          # batch rows per core
    TB = Bc // P                 # batch tiles per core
    NB = min(256, Bc)            # matmul moving free dim
    NN = Bc // NB                # batch n-tiles
    TPN = NB // P                # 128-tiles per n-tile
    NKC = _RWF // P              # K chunks (14)
    NM1 = H1 // P                # 8
    NM2 = H2 // P                # 4
    rg = [list(range(ncore))]

    NQ = cfg.get("swdge_queues", 4)
    nc = bacc.Bacc(num_devices=ncore, num_swdge_queues=NQ)

    idxT = nc.dram_tensor("idxT", [P, TB * F_CAT], I32, kind="ExternalInput")
    cfT = nc.dram_tensor("cfT", [P, TB * F_CONT], FP16, kind="ExternalInput")
    bigt = nc.dram_tensor("bigt", [F_CAT * Vv, _EW], FP16, kind="ExternalInput")
    w1 = nc.dram_tensor("w1", [_RWF, H1], FP16, kind="ExternalInput")
    wsel = nc.dram_tensor("wsel", [_RWF, _EW], FP16, kind="ExternalInput")
    w2 = nc.dram_tensor("w2", [H1, H2], FP16, kind="ExternalInput")
    w3 = nc.dram_tensor("w3", [P, NM2], FP16, kind="ExternalInput")
    bnp = nc.dram_tensor("bnp", [P, 3 * NM1 + 3 * NM2 + 1], F32, kind="ExternalInput")
    ident = nc.dram_tensor("ident", [P, P], FP16, kind="ExternalInput")
    out = nc.dram_tensor("out", [2, Bc], F32, kind="ExternalOutput")

    with tile.TileContext(nc) as tc:
        with (
            tc.tile_pool(name="const", bufs=1) as cpool,
            tc.tile_pool(name="big", bufs=1) as bpool,
            tc.tile_pool(name="s2p", bufs=2) as s2pool,
            tc.tile_pool(name="psmm", bufs=3, space="PSUM") as psmm,
            tc.tile_pool(name="psel", bufs=1, space="PSUM") as psel,
            tc.tile_pool(name="pz", bufs=1, space="PSUM") as pz,
            tc.tile_pool(name="ptt", bufs=2, space="PSUM") as ptt,
            tc.tile_pool(name="dram", bufs=1, space="DRAM") as dpool,
        ):
            # ---- constants (batch-dependent inputs first on the sync queue
            # so gathers can start immediately; weights trickle in on the
            # scalar hwdge queue) ----
            idx_sb = cpool.tile([P, TB * F_CAT], I32, tag="idxT")
            nc.sync.dma_start(out=idx_sb[:, 0 : 2 * F_CAT], in_=idxT[:, 0 : 2 * F_CAT])
            nc.sync.dma_start(
                out=idx_sb[:, 2 * F_CAT :], in_=idxT[:, 2 * F_CAT :]
            )
            cf_sb = cpool.tile([P, TB * F_CONT], FP16, tag="cfT")
            nc.sync.dma_start(out=cf_sb[:], in_=cfT[:])
            bnsb = cpool.tile([P, 3 * NM1 + 3 * NM2 + 1], F32, tag="bnp")
            nc.sync.dma_start(out=bnsb[:], in_=bnp[:])
            w1sb = []
            for k in range(NKC):
                t = cpool.tile([P, H1], FP16, tag=f"w1_{k}")
                nc.scalar.dma_start(out=t[:], in_=w1[k * P : (k + 1) * P, :])
                w1sb.append(t)
            wselsb = []
            for k in range(NKC):
                t = cpool.tile([P, _EW], FP16, tag=f"wsel_{k}")
                nc.scalar.dma_start(out=t[:], in_=wsel[k * P : (k + 1) * P, :])
                wselsb.append(t)
            w2sb = []
            for k in range(NM1):
                t = cpool.tile([P, H2], FP16, tag=f"w2_{k}")
                nc.scalar.dma_start(out=t[:], in_=w2[k * P : (k + 1) * P, :])
                w2sb.append(t)
            w3sb = cpool.tile([P, NM2], FP16, tag="w3")
            nc.scalar.dma_start(out=w3sb[:], in_=w3[:])
            eps_t = cpool.tile([P, 1], F32, tag="eps")
            nc.vector.memset(eps_t[:], BN_EPS)
            halves = cpool.tile([D, 1], FP16, tag="halves")
            nc.vector.memset(halves[:], 0.5)
            identsb = cpool.tile([P, P], FP16, tag="ident")
            nc.sync.dma_start(out=identsb[:], in_=ident[:])

            b1c = bnsb[:, 0:NM1]
            g1c = bnsb[:, NM1 : 2 * NM1]
            be1c = bnsb[:, 2 * NM1 : 3 * NM1]
            o2 = 3 * NM1
            b2c = bnsb[:, o2 : o2 + NM2]
            g2c = bnsb[:, o2 + NM2 : o2 + 2 * NM2]
            be2c = bnsb[:, o2 + 2 * NM2 : o2 + 3 * NM2]
            bias_col = bnsb[:, o2 + 3 * NM2 : o2 + 3 * NM2 + 1]

            # cont squared features, all tiles at once
            cfsq = cpool.tile([P, TB * F_CONT], FP16, tag="cfsq")
            nc.vector.tensor_tensor(out=cfsq[:], in0=cf_sb[:], in1=cf_sb[:], op=OP.mult)

            # ---- persistent activations ----
            xtn = [
                bpool.tile([P, NKC, NB], FP16, tag=f"xtn_{n}", name=f"xtn_{n}")
                for n in range(NN)
            ]
            h1t = [bpool.tile([P, Bc], FP16, tag=f"h1_{m}", name=f"h1_{m}") for m in range(NM1)]
            h2t = [bpool.tile([P, Bc], FP16, tag=f"h2_{m}", name=f"h2_{m}") for m in range(NM2)]

            acc1 = bpool.tile([P, NM1 * NN], F32, tag="acc1")
            acc1s = bpool.tile([P, NM1 * NN], F32, tag="acc1s")
            acc2 = bpool.tile([P, NM2 * NN], F32, tag="acc2")
            acc2s = bpool.tile([P, NM2 * NN], F32, tag="acc2s")
            scrh = bpool.tile([P, NB], FP16, tag="scrh")
            fmsb = bpool.tile([1, Bc], F32, tag="fmsb")
            zrow = bpool.tile([1, Bc], F32, tag="zrow")
            outp = bpool.tile([1, Bc], F32, tag="outp")
            outn = bpool.tile([1, Bc], F32, tag="outn")

            # gather row buffers; pad+cont regions zeroed once (pad columns
            # hit zero weight rows, but must stay finite for fp16 matmul)
            NRB = 6
            rows_bufs = [
                bpool.tile([P, _RWF], FP16, tag=f"rows{j}", name=f"rows{j}")
                for j in range(NRB)
            ]
            for j in range(NRB):
                nc.vector.memset(rows_bufs[j][:, _RWG:_RWF], 0.0)

            # ---- gather + transpose + per-n-tile compute, interleaved ----
            gq = [0]

            def emit_tile(t):
                rows = rows_bufs[t % NRB]
                for f in range(F_CAT):
                    inst = nc.gpsimd.indirect_dma_start(
                        out=rows[:, f * _EW : (f + 1) * _EW],
                        out_offset=None,
                        in_=bigt[:],
                        in_offset=bass.IndirectOffsetOnAxis(
                            ap=idx_sb[:, t * F_CAT + f : t * F_CAT + f + 1], axis=0
                        ),
                    )
                    if NQ > 1:
                        inst.ins.queue = f"qPoolDynamic{(gq[0] % NQ) or ''}"
                        gq[0] += 1
                nc.vector.tensor_copy(
                    out=rows[:, _CFO:_CFE],
                    in_=cf_sb[:, t * F_CONT : (t + 1) * F_CONT],
                )
                nc.vector.tensor_copy(
                    out=rows[:, _CQO:_CQE],
                    in_=cfsq[:, t * F_CONT : (t + 1) * F_CONT],
                )
                n, tp = t // TPN, t % TPN
                for k in range(NKC):
                    pst = ptt.tile([P, P], FP16, tag="tt")
                    nc.tensor.transpose(
                        out=pst[:], in_=rows[:, k * P : (k + 1) * P], identity=identsb[:]
                    )
                    dst = xtn[n][:, k, tp * P : (tp + 1) * P]
                    if k % 2 == 0:
                        nc.scalar.activation(out=dst, in_=pst[:], func=AF.Copy)
                    else:
                        nc.vector.tensor_copy(out=dst, in_=pst[:])

            def emit_ntile(n):
                # layer-1 matmuls for this n-tile
                for m in range(NM1):
                    ps = psmm.tile([P, NB], F32, tag="mm")
                    for k in range(NKC):
                        nc.tensor.matmul(
                            out=ps[:],
                            lhsT=w1sb[k][:, m * P : (m + 1) * P],
                            rhs=xtn[n][:, k, :],
                            start=(k == 0),
                            stop=(k == NKC - 1),
                        )
                    j = m * NN + n
                    nc.scalar.activation(
                        out=h1t[m][:, n * NB : (n + 1) * NB], in_=ps[:],
                        func=AF.Identity, bias=b1c[:, m : m + 1],
                    )
                    nc.vector.tensor_reduce(
                        out=acc1[:, j : j + 1], in_=ps[:], axis=AX.X, op=OP.add
                    )
                    nc.vector.tensor_tensor(
                        out=scrh[:],
                        in0=h1t[m][:, n * NB : (n + 1) * NB],
                        in1=h1t[m][:, n * NB : (n + 1) * NB],
                        op=OP.mult,
                    )
                    nc.vector.tensor_reduce(
                        out=acc1s[:, j : j + 1], in_=scrh[:], axis=AX.X, op=OP.add
                    )

                # FM selection stack: SEL = s(64) x NB (cont folded via wsel)
                sel = psel.tile([D, NB], F32, tag="sel")
                for k in range(NKC):
                    nc.tensor.matmul(
                        out=sel[:],
                        lhsT=wselsb[k][:, 0:D],
                        rhs=xtn[n][:, k, :],
                        start=(k == 0),
                        stop=(k == NKC - 1),
                    )
                s2t = s2pool.tile([D, NB], FP16, tag="s2")
                nc.scalar.activation(out=s2t[:], in_=sel[:], func=AF.Square)
                # zfm = (first_total - 0.5*qsum) + 0.5*sum_d s^2, all as M=1
                # matmuls accumulating at PSUM partition 0
                zfm = pz.tile([1, NB], F32, tag="zfm")
                for k in range(NKC):
                    nc.tensor.matmul(
                        out=zfm[:],
                        lhsT=wselsb[k][:, D : D + 1],
                        rhs=xtn[n][:, k, :],
                        start=(k == 0),
                        stop=False,
                    )
                nc.tensor.matmul(
                    out=zfm[:], lhsT=halves[:], rhs=s2t[:], start=False, stop=True
                )
                nsl = slice(n * NB, (n + 1) * NB)
                nc.vector.tensor_copy(out=fmsb[0:1, nsl], in_=zfm[:])

            for t in range(TB):
                emit_tile(t)
                if t % TPN == TPN - 1:
                    emit_ntile(t // TPN)

            # ---- BN1 stats ----
            st1 = bpool.tile([P, 2 * NM1], F32, tag="st1")
            nc.vector.tensor_reduce(
                out=st1[:, :NM1],
                in_=acc1[:].rearrange("p (m n) -> p m n", n=NN),
                axis=AX.X, op=OP.add,
            )
            nc.vector.tensor_reduce(
                out=st1[:, NM1:],
                in_=acc1s[:].rearrange("p (m n) -> p m n", n=NN),
                axis=AX.X, op=OP.add,
            )
            st1i = dpool.tile([P, 2 * NM1], F32, tag="st1i")
            st1o = dpool.tile([P, 2 * NM1], F32, tag="st1o")
            nc.gpsimd.dma_start(out=st1i[:], in_=st1[:])
            nc.gpsimd.collective_compute(
                "AllReduce", OP.add, replica_groups=rg,
                ins=[st1i[:].opt()], outs=[st1o[:].opt()],
            )
            gst1 = bpool.tile([P, 2 * NM1], F32, tag="gst1")
            nc.gpsimd.dma_start(out=gst1[:], in_=st1o[:])

            mu1 = bpool.tile([P, NM1], F32, tag="mu1")
            var1 = bpool.tile([P, NM1], F32, tag="var1")
            a1 = bpool.tile([P, NM1], F32, tag="a1")
            bp1 = bpool.tile([P, NM1], F32, tag="bp1")
            inv_b = 1.0 / Bfull
            nc.vector.tensor_scalar(
                out=mu1[:], in0=gst1[:, :NM1], scalar1=inv_b, scalar2=None, op0=OP.mult
            )
            nc.vector.tensor_tensor(out=var1[:], in0=mu1[:], in1=mu1[:], op=OP.mult)
            nc.vector.tensor_scalar(
                out=a1[:], in0=gst1[:, NM1:], scalar1=inv_b, scalar2=None, op0=OP.mult
            )
            nc.vector.tensor_tensor(out=var1[:], in0=a1[:], in1=var1[:], op=OP.subtract)
            nc.scalar.activation(
                out=var1[:], in_=var1[:], func=AF.Sqrt, bias=eps_t[:, 0:1]
            )
            nc.vector.reciprocal(out=var1[:], in_=var1[:])
            nc.vector.tensor_tensor(out=a1[:], in0=g1c, in1=var1[:], op=OP.mult)
            nc.vector.tensor_tensor(out=bp1[:], in0=mu1[:], in1=a1[:], op=OP.mult)
            nc.vector.tensor_tensor(out=bp1[:], in0=be1c, in1=bp1[:], op=OP.subtract)

            # ---- relu1 + layer 2, pipelined per n ----
            for n in range(NN):
                for m in range(NM1):
                    nc.scalar.activation(
                        out=h1t[m][:, n * NB : (n + 1) * NB],
                        in_=h1t[m][:, n * NB : (n + 1) * NB],
                        func=AF.Relu,
                        scale=a1[:, m : m + 1], bias=bp1[:, m : m + 1],
                    )
                for m in range(NM2):
                    ps = psmm.tile([P, NB], F32, tag="mm")
                    for k in range(NM1):
                        nc.tensor.matmul(
                            out=ps[:],
                            lhsT=w2sb[k][:, m * P : (m + 1) * P],
                            rhs=h1t[k][:, n * NB : (n + 1) * NB],
                            start=(k == 0),
                            stop=(k == NM1 - 1),
                        )
                    j = m * NN + n
                    nc.scalar.activation(
                        out=h2t[m][:, n * NB : (n + 1) * NB], in_=ps[:],
                        func=AF.Identity, bias=b2c[:, m : m + 1],
                    )
                    nc.vector.tensor_reduce(
                        out=acc2[:, j : j + 1], in_=ps[:], axis=AX.X, op=OP.add
                    )
                    nc.vector.tensor_tensor(
                        out=scrh[:],
                        in0=h2t[m][:, n * NB : (n + 1) * NB],
                        in1=h2t[m][:, n * NB : (n + 1) * NB],
                        op=OP.mult,
                    )
                    nc.vector.tensor_reduce(
                        out=acc2s[:, j : j + 1], in_=scrh[:], axis=AX.X, op=OP.add
                    )

            # ---- BN2 ----
            st2 = bpool.tile([P, 2 * NM2], F32, tag="st2")
            nc.vector.tensor_reduce(
                out=st2[:, :NM2],
                in_=acc2[:].rearrange("p (m n) -> p m n", n=NN),
                axis=AX.X, op=OP.add,
            )
            nc.vector.tensor_reduce(
                out=st2[:, NM2:],
                in_=acc2s[:].rearrange("p (m n) -> p m n", n=NN),
                axis=AX.X, op=OP.add,
            )
            st2i = dpool.tile([P, 2 * NM2], F32, tag="st2i")
            st2o = dpool.tile([P, 2 * NM2], F32, tag="st2o")
            nc.gpsimd.dma_start(out=st2i[:], in_=st2[:])
            nc.gpsimd.collective_compute(
                "AllReduce", OP.add, replica_groups=rg,
                ins=[st2i[:].opt()], outs=[st2o[:].opt()],
            )
            gst2 = bpool.tile([P, 2 * NM2], F32, tag="gst2")
            nc.gpsimd.dma_start(out=gst2[:], in_=st2o[:])

            mu2 = bpool.tile([P, NM2], F32, tag="mu2")
            var2 = bpool.tile([P, NM2], F32, tag="var2")
            a2 = bpool.tile([P, NM2], F32, tag="a2")
            bp2 = bpool.tile([P, NM2], F32, tag="bp2")
            nc.vector.tensor_scalar(
                out=mu2[:], in0=gst2[:, :NM2], scalar1=inv_b, scalar2=None, op0=OP.mult
            )
            nc.vector.tensor_tensor(out=var2[:], in0=mu2[:], in1=mu2[:], op=OP.mult)
            nc.vector.tensor_scalar(
                out=a2[:], in0=gst2[:, NM2:], scalar1=inv_b, scalar2=None, op0=OP.mult
            )
            nc.vector.tensor_tensor(out=var2[:], in0=a2[:], in1=var2[:], op=OP.subtract)
            nc.scalar.activation(
                out=var2[:], in_=var2[:], func=AF.Sqrt, bias=eps_t[:, 0:1]
            )
            nc.vector.reciprocal(out=var2[:], in_=var2[:])
            nc.vector.tensor_tensor(out=a2[:], in0=g2c, in1=var2[:], op=OP.mult)
            nc.vector.tensor_tensor(out=bp2[:], in0=mu2[:], in1=a2[:], op=OP.mult)
            nc.vector.tensor_tensor(out=bp2[:], in0=be2c, in1=bp2[:], op=OP.subtract)

            # ---- relu2 + layer 3 + sigmoid + output, per n ----
            for n in range(NN):
                for m in range(NM2):
                    nc.scalar.activation(
                        out=h2t[m][:, n * NB : (n + 1) * NB],
                        in_=h2t[m][:, n * NB : (n + 1) * NB],
                        func=AF.Relu,
                        scale=a2[:, m : m + 1], bias=bp2[:, m : m + 1],
                    )
                zz = pz.tile([1, NB], F32, tag="zz")
                for c in range(NM2):
                    nc.tensor.matmul(
                        out=zz[:],
                        lhsT=w3sb[:, c : c + 1],
                        rhs=h2t[c][:, n * NB : (n + 1) * NB],
                        start=(c == 0),
                        stop=(c == NM2 - 1),
                    )
                nsl = slice(n * NB, (n + 1) * NB)
                nc.vector.tensor_tensor(
                    out=zrow[0:1, nsl], in0=zz[:], in1=fmsb[0:1, nsl], op=OP.add
                )
            nc.scalar.activation(
                out=outp[:], in_=zrow[:],
                func=AF.Sigmoid, bias=bias_col[0:1, :],
            )
            nc.vector.tensor_scalar(
                out=outn[:], in0=outp[:], scalar1=-1.0, scalar2=1.0,
                op0=OP.mult, op1=OP.add,
            )
            nc.sync.dma_start(out=out[1:2, :], in_=outp[:])
            nc.sync.dma_start(out=out[0:1, :], in_=outn[:])

    return nc


def _prep_shared(inputs, cfg):
    """Host-side parameter prep (batch-independent). Returns dict of arrays
    shared by all cores."""
    Vv = cfg["V"]
    f32 = np.float32
    f16 = np.float16
    cat_t1 = np.asarray(inputs["cat_t1"], f32)          # [26, V]
    cat_t2 = np.asarray(inputs["cat_t2"], f32)          # [26, V, 64]
    cont_t1 = np.asarray(inputs["cont_t1"], f32)        # [13]
    cont_t2 = np.asarray(inputs["cont_t2"], f32)        # [13, 64]
    W1 = np.asarray(inputs["W1"], f32)                  # [2496, 1024]
    W2 = np.asarray(inputs["W2"], f32)
    W3 = np.asarray(inputs["W3"], f32)                  # [512, 1]
    b1 = np.asarray(inputs["b1"], f32)
    g1 = np.asarray(inputs["g1"], f32)
    be1 = np.asarray(inputs["be1"], f32)
    b2 = np.asarray(inputs["b2"], f32)
    g2 = np.asarray(inputs["g2"], f32)
    be2 = np.asarray(inputs["be2"], f32)
    b3 = np.asarray(inputs["b3"], f32)
    bias = np.asarray(inputs["bias"], f32)

    t2f = cat_t2.reshape(F_CAT * Vv, D).astype(f16)
    bigt = np.empty((F_CAT * Vv, _EW), f16)
    bigt[:, :D] = t2f
    bigt[:, D] = cat_t1.reshape(F_CAT * Vv)
    # row sum-of-squares of the fp16 embeddings (matches device arithmetic)
    bigt[:, D + 1] = (t2f.astype(f32) ** 2).sum(axis=1)

    ncat = F_CAT * D  # 1664
    W1eff = np.einsum("fd,fdh->fh", cont_t2, W1[ncat:].reshape(F_CONT, D, H1))
    # permute W1 rows to the gathered-row layout k' = f*66 + e; t1/sumsq and
    # cont-squared rows are zero, cont rows folded through cont_t2
    w1p = np.zeros((_RWF, H1), f32)
    w1p[:_RWG].reshape(F_CAT, _EW, H1)[:, :D, :] = W1[:ncat].reshape(F_CAT, D, H1)
    w1p[_CFO:_CFE] = W1eff

    # FM selection matrix: cols 0..63 give s = sum_f E (cont folded via
    # cont_t2); col 64 gives the linear fm part first_total - 0.5*qsum
    wselp = np.zeros((_RWF, _EW), f32)
    wv = wselp[:_RWG].reshape(F_CAT, _EW, _EW)
    for e in range(D):
        wv[:, e, e] = 1.0
    wv[:, D, D] = 1.0           # first-order totals
    wv[:, D + 1, D] = -0.5      # -0.5 * sum-of-squares totals
    wselp[_CFO:_CFE, :D] = cont_t2          # s_cont = cf @ cont_t2
    wselp[_CFO:_CFE, D] = cont_t1           # first-order cont
    wselp[_CQO:_CQE, D] = -0.5 * (cont_t2**2).sum(axis=1)  # -0.5 * qct

    NM1n, NM2n = H1 // _P, H2 // _P
    bnpa = np.zeros((_P, 3 * NM1n + 3 * NM2n + 1), f32)
    bnpa[:, 0:NM1n] = b1.reshape(NM1n, _P).T
    bnpa[:, NM1n : 2 * NM1n] = g1.reshape(NM1n, _P).T
    bnpa[:, 2 * NM1n : 3 * NM1n] = be1.reshape(NM1n, _P).T
    o2 = 3 * NM1n
    bnpa[:, o2 : o2 + NM2n] = b2.reshape(NM2n, _P).T
    bnpa[:, o2 + NM2n : o2 + 2 * NM2n] = g2.reshape(NM2n, _P).T
    bnpa[:, o2 + 2 * NM2n : o2 + 3 * NM2n] = be2.reshape(NM2n, _P).T
    bnpa[:, o2 + 3 * NM2n] = float(bias[0]) + float(b3[0])

    return {
        "ident": np.eye(_P, dtype=f16),
        "bigt": bigt,
        "w1": w1p.astype(f16),
        "wsel": wselp.astype(f16),
        "w2": W2.astype(f16),
        "w3": W3[:, 0].reshape(NM2n, _P).T.astype(f16).copy(),
        "bnp": bnpa,
    }


def _prep_in_maps(inputs, cfg):
    """Build the per-core input maps (shard batch, replicate params)."""
    ncore = cfg["n_cores"]
    Vv = cfg["V"]
    Bc = cfg["B"] // ncore
    TB = Bc // _P
    shared = _prep_shared(inputs, cfg)
    cat = np.asarray(inputs["cat_feats"]).astype(np.int32)
    cont = np.asarray(inputs["cont_feats"], np.float32).astype(np.float16)
    idxg = cat + (np.arange(F_CAT, dtype=np.int32) * Vv)[None, :]
    in_maps = []
    for c in range(ncore):
        m = dict(shared)
        # transpose batch-sharded inputs to [128, TB*F] (partition-contiguous)
        ic = idxg[c * Bc : (c + 1) * Bc].reshape(TB, _P, F_CAT)
        m["idxT"] = np.ascontiguousarray(ic.transpose(1, 0, 2)).reshape(_P, TB * F_CAT)
        cc = cont[c * Bc : (c + 1) * Bc].reshape(TB, _P, F_CONT)
        m["cfT"] = np.ascontiguousarray(cc.transpose(1, 0, 2)).reshape(_P, TB * F_CONT)
        in_maps.append(m)
    return in_maps


def _unshard(results, cfg):
    ncore = cfg["n_cores"]
    outs = []
    for c in range(ncore):
        a = results[c]["out"]  # [2, Bc]; column b = batch row b of the shard
        outs.append(np.stack([a[0], a[1]], axis=1))
    return np.concatenate(outs, axis=0)


_CACHE = {}


def _get_program(cfg_key):
    if cfg_key not in _CACHE:
        cfg = dict(B=cfg_key[0], V=cfg_key[1], n_cores=cfg_key[2])
        nc = _build_program(cfg)
        nc.finalize()
        _CACHE[cfg_key] = nc
    return _CACHE[cfg_key]


def run(inputs, trace=False, cfg=None):
    from concourse import bass_utils

    cfg = cfg or CFG_FULL
    nc = _get_program((cfg["B"], cfg["V"], cfg["n_cores"]))
    in_maps = _prep_in_maps(inputs, cfg)
    res = bass_utils.run_bass_kernel_spmd(
        nc, in_maps, core_ids=list(range(cfg["n_cores"])), trace=trace
    )
    return _unshard(res.results, cfg), res


def kernel(**inputs) -> np.ndarray:
    out, _ = run(inputs, trace=False)
    return out
